# revision 24
# baseline (speedup 1.0000x reference)
"""Trainium2 Bass kernel for dual-score multi-head self-attention.

Reference computation (per batch b):
    q   = x @ Wq.T + bq          k = x @ Wk.T + bk
    v   = x @ Wv.T + bv          pos = pos_emb @ Wp.T + bp
    scores = (q k^T + q pos^T) / sqrt(dh)   (per head)
    out = softmax(scores) @ v, merged heads, @ Wo.T + bo

Algebraic folds:
  * q k^T + q pos^T == q (k+pos)^T; k+pos accumulates in one PSUM tile.
  * v's bias commutes through softmax (rows sum to 1): bo' = bo + Wo @ bv.
  * softmax denominators ride along in the attention*V matmul via ones
    columns appended to V (even heads [v|ones], odd heads [ones|v] so
    every DVE op in the normalization runs at a legal base partition).
  * q is pre-scaled by ALPHA = 128*log2(e)/8 so scores arrive in
    "128-scaled log2" units: the DVE exp tiles consume them directly via
    a fused custom op (corrected Schraudolph), the ScalarE exp tiles undo
    the scale inside the activation's scale factor.

Exp is split across ScalarE (activation table) and DVE (custom fused op:
bits = c2*(|frac_128(s)| - h)^2 + s + K, rounded into bf16's u16 bit
pattern) to break the ScalarE bottleneck; softmax normalization muls run
on GPSIMD.

Sharding: batch parallel, one batch per core, weights replicated, no
collectives. See bottom for host-side prep.
"""

import numpy as np

try:
    import concourse.bass as bass  # noqa: F401
except ImportError:  # pragma: no cover - container default path
    import sys

    for p in ("/opt/trn_rl_repo", "/root/.axon_site/_ro/trn_rl_repo"):
        if p not in sys.path:
            sys.path.insert(0, p)

import ml_dtypes

import concourse.bass as bass
import concourse.mybir as mybir
import concourse.tile as tile
import concourse.dve_ops as dve_ops
from concourse import bacc
from concourse.bass import ts
from concourse.bass_utils import run_bass_kernel_spmd
from concourse.dve_spec import Spec, Src0, Src1, C0, C1, C2, lower, Bin, AluOp, sq
from concourse.dve_ops import DveOp, OPS, DveOpSpec, has_src1

P = 128
T = 2048
D = 512
H = 8
DH = 64
B = 8
MT = D // P  # 4 feature tiles
KT = T // P  # 16 kpos tiles
QC = 512  # q chunk (one PSUM bank of f32)
NQC = T // QC

BF = mybir.dt.bfloat16
F32 = mybir.dt.float32
U16 = mybir.dt.uint16
ADD = mybir.AluOpType.add
MULT = mybir.AluOpType.mult
EXP = mybir.ActivationFunctionType.Exp

N_CORES = 8

# ---------------------------------------------------------------------------
# Custom DVE exp op: corrected Schraudolph.
#   in0 = s' (scores pre-scaled by ALPHA), out bits (u16, reinterpreted bf16)
#   bits = c2*(|s' - RN_128(s')| - h)^2 + (s' + K)
# Calibrated: rms rel err 0.25%, max 0.55% vs exp (incl. bf16 rounding).
EXP_H = 66.124491335875774
EXP_C2 = 0.0025442885269545651
EXP_K = 16244.943801046942
EXP_MAGIC = float(1.5 * 2.0**30)
ALPHA = float(128.0 * np.log2(np.e) / 8.0)  # q pre-scale per raw score unit
ACT_SCALE = float(1.0 / (8.0 * ALPHA))  # ScalarE: exp(s' * ACT_SCALE)

# Exp tiles are split per head: head-even half on ScalarE, head-odd half
# on the DVE custom op, concurrently. Halving the exp instruction length
# halves the score->AV latency, which is what lets the lag-2 software
# pipeline below run gapless on the PE.


def _exp_ref(in0, in1, s0, s1, imm2):
    in0 = in0.astype(np.float32)
    w = (in0 + np.float32(s1)).astype(np.float32)
    r = (w - np.float32(s1)).astype(np.float32)
    af = np.abs(in0 - r).astype(np.float32)
    d = (af - np.float32(s0)).astype(np.float32)
    return ((d * d * np.float32(imm2)) + (in0 + in1)).astype(np.float32)


def _register_exp_op():
    for op in OPS:
        if op.name == "EXP_SCH_ANT":
            return op
    w = Src0 + C1
    r = w - C1
    af = Bin(AluOp.ABSOLUTE_DIFF, Src0, r)
    d = af - C0
    body = sq(d) * C2 + (Src0 + Src1)
    spec = Spec(body=body, reference=_exp_ref)
    op = DveOp("EXP_SCH_ANT", spec, subdim=False, uops_sha={})
    OPS.append(op)
    dve_ops._SUB_OPCODE_FOR_NAME[op.name] = dve_ops._CUSTOM_DVE_ROW_BASE + len(OPS) - 1
    dve_ops.CUSTOM_DVE_SPECS[op.name] = spec
    opc = dve_ops.get_dve_sub_opcode(op.name)
    for ver in ("v3", "v4"):
        uops = lower(spec, ver=ver)
        comp = DveOpSpec(name=op.name, opcode=opc, uops=uops, rd1_en=has_src1(spec))
        op.uops_sha[ver] = comp.sha(ver)
    return op


EXP_OP = _register_exp_op()


def _emit(ctx, tc, io):
    nc = tc.nc

    # ---- persistent SBUF tensors -------------------------------------
    const_pool = ctx.enter_context(tc.tile_pool(name="const", bufs=1))

    def single(name, shape, dtype):
        return const_pool.tile(shape, dtype, name=name, tag=name)

    w_sb = {w: single(f"w_{w}", [P, MT, D], BF) for w in
            ("wq", "wk", "wp", "wv", "wo")}
    b_sb = {b: single(f"b_{b}", [P, MT], F32) for b in ("bq", "bkp", "bob")}
    xT_sb = single("xT_sb", [P, MT, T], BF)
    posT_sb = single("posT_sb", [P, MT, T], BF)
    qT_sb = [single(f"qT{m}", [P, T], BF) for m in range(MT)]
    kT_sb = [single(f"kT{m}", [P, T], BF) for m in range(MT)]
    ctx_sb = [single(f"ctxT{m}", [P, T], BF) for m in range(MT)]
    v_sb = [single(f"v{kt}", [P, H, P], BF) for kt in range(KT)]
    # elementwise K operand for the custom DVE exp ([P,1] broadcast Src1
    # wedges the DVE on this stack, so spend the SBUF on a full tile)
    kconst_sb = single("kconst", [P, QC], F32)
    # partition-swap matrix (i <-> i+64) for the tail's PE-based
    # denominator extraction
    eyeshift_sb = single("eyeshift", [P, P], BF)

    # ---- input DMAs: one transfer per tensor (chunked along T for x/pos
    # so compute starts early); both HWDGE queues carry ~equal bytes in
    # first-need order (q-proj -> k'-proj -> v-proj -> rest).
    nc.sync.dma_start(out=xT_sb[:, :, 0:QC], in_=io["xT"][:, :, 0:QC])
    nc.scalar.dma_start(out=w_sb["wq"], in_=io["wqT"])
    nc.sync.dma_start(out=posT_sb[:, :, 0:QC], in_=io["posT"][:, :, 0:QC])
    nc.scalar.dma_start(out=w_sb["wk"], in_=io["wkT"])
    nc.sync.dma_start(out=w_sb["wv"], in_=io["wvT"])
    nc.scalar.dma_start(out=w_sb["wp"], in_=io["wpT"])
    nc.scalar.dma_start(out=b_sb["bq"], in_=io["bq"])
    nc.scalar.dma_start(out=b_sb["bkp"], in_=io["bkp"])
    nc.sync.dma_start(out=xT_sb[:, :, QC:T], in_=io["xT"][:, :, QC:T])
    nc.scalar.dma_start(out=posT_sb[:, :, QC:T], in_=io["posT"][:, :, QC:T])
    nc.sync.dma_start(out=w_sb["wo"], in_=io["woT"])
    nc.scalar.dma_start(out=b_sb["bob"], in_=io["bob"])
    nc.scalar.dma_start(out=eyeshift_sb, in_=io["eyeshift"])

    nc.gpsimd.memset(kconst_sb, EXP_K)
    for kt in range(KT):
        # only the ones-regions: keeps the v drains (disjoint slices)
        # independent of the memsets under subtile dep tracking
        nc.gpsimd.memset(v_sb[kt][:, 0 : H : 2, DH:P], 1.0)
        nc.gpsimd.memset(v_sb[kt][:, 1 : H : 2, 0:DH], 1.0)

    # ---- pools --------------------------------------------------------
    ps_proj = ctx.enter_context(tc.tile_pool(name="ps_proj", bufs=2, space="PSUM"))
    ps_sc = ctx.enter_context(tc.tile_pool(name="ps_sc", bufs=4, space="PSUM"))
    ps_av = ctx.enter_context(tc.tile_pool(name="ps_av", bufs=2, space="PSUM"))
    expp = ctx.enter_context(tc.tile_pool(name="expp", bufs=8))
    recp = ctx.enter_context(tc.tile_pool(name="recp", bufs=4))
    stagep = ctx.enter_context(tc.tile_pool(name="stagep", bufs=4))
    outp = ctx.enter_context(tc.tile_pool(name="outp", bufs=6))

    # prime the ScalarE exp table set (~2.7us one-time load) during the
    # DMA phase instead of on the first real score tile
    warm = recp.tile([P, 1], F32, name="warm", tag="warm")
    nc.vector.memset(warm, 0.0)
    nc.scalar.activation(out=warm, in_=warm, func=EXP, scale=1.0)

    # ---- v projection (natural layout, no bias) ----------------------
    def emit_v_proj(tts=None):
        for tt in tts if tts is not None else range(KT):
            ps = ps_proj.tile([P, D], F32, name=f"vps{tt}", tag="ps_proj")
            for k in range(MT):
                nc.tensor.matmul(
                    ps,
                    lhsT=xT_sb[:, k, ts(tt, P)],
                    rhs=w_sb["wv"][:, k, :],
                    start=(k == 0),
                    stop=(k == MT - 1),
                )
            ps_h = ps.rearrange("p (h d) -> p h d", h=H)
            nc.vector.tensor_copy(out=v_sb[tt][:, 0:H:2, 0:DH], in_=ps_h[:, 0:H:2, :])
            nc.vector.tensor_copy(out=v_sb[tt][:, 1:H:2, DH:P], in_=ps_h[:, 1:H:2, :])

    # ---- q / k' projections for one feature tile m -------------------
    # q is scaled by ALPHA so the score matmul emits pre-scaled scores.
    def emit_qk_proj(m, qcs=None):
        for qc in qcs if qcs is not None else range(NQC):
            ps = ps_proj.tile([P, QC], F32, name=f"qps{m}_{qc}", tag="ps_proj")
            for k in range(MT):
                nc.tensor.matmul(
                    ps,
                    lhsT=w_sb["wq"][:, k, ts(m, P)],
                    rhs=xT_sb[:, k, ts(qc, QC)],
                    start=(k == 0),
                    stop=(k == MT - 1),
                )
            nc.vector.tensor_scalar(
                out=qT_sb[m][:, ts(qc, QC)],
                in0=ps,
                scalar1=b_sb["bq"][:, m : m + 1],
                scalar2=ALPHA,
                op0=ADD,
                op1=MULT,
            )
            ps2 = ps_proj.tile([P, QC], F32, name=f"kps{m}_{qc}", tag="ps_proj")
            for k in range(MT):
                nc.tensor.matmul(
                    ps2,
                    lhsT=w_sb["wk"][:, k, ts(m, P)],
                    rhs=xT_sb[:, k, ts(qc, QC)],
                    start=(k == 0),
                    stop=False,
                )
            for k in range(MT):
                nc.tensor.matmul(
                    ps2,
                    lhsT=w_sb["wp"][:, k, ts(m, P)],
                    rhs=posT_sb[:, k, ts(qc, QC)],
                    start=False,
                    stop=(k == MT - 1),
                )
            nc.vector.tensor_scalar(
                out=kT_sb[m][:, ts(qc, QC)],
                in0=ps2,
                scalar1=b_sb["bkp"][:, m : m + 1],
                scalar2=None,
                op0=ADD,
            )

    # ---- attention for head pair p (heads 2p, 2p+1) ------------------
    # Lag-2 software pipeline: the AV pair for kt is emitted two score
    # iterations later, giving the exps (ScalarE half + DVE half, run
    # concurrently) two full PE slots of latency headroom.
    AV_LAG = 2

    def emit_attention(p, on_qc_done=None, pre_kt=None):
        for qc in range(NQC):
            psA = ps_av.tile([P, QC], F32, name=f"avA{p}_{qc}", tag="av")
            psB = ps_av.tile([P, QC], F32, name=f"avB{p}_{qc}", tag="av")
            e_tiles = {}

            def emit_av(kt):
                eA, eB = e_tiles.pop(kt)
                nc.tensor.matmul(
                    psA,
                    lhsT=v_sb[kt][:, 2 * p, :],
                    rhs=eA,
                    start=(kt == 0),
                    stop=(kt == KT - 1),
                    skip_group_check=True,
                )
                nc.tensor.matmul(
                    psB,
                    lhsT=v_sb[kt][:, 2 * p + 1, :],
                    rhs=eB,
                    start=(kt == 0),
                    stop=(kt == KT - 1),
                    skip_group_check=True,
                )

            for kt in range(KT):
                if pre_kt is not None:
                    pre_kt(qc, kt)
                sA = ps_sc.tile([P, QC], F32, name=f"scA{p}_{qc}_{kt}", tag="sc")
                sB = ps_sc.tile([P, QC], F32, name=f"scB{p}_{qc}_{kt}", tag="sc")
                # scores^T = k'^T.T @ q^T, two heads packed via row tiling
                nc.tensor.matmul(
                    sA,
                    lhsT=kT_sb[p][0:DH, ts(kt, P)],
                    rhs=qT_sb[p][0:DH, ts(qc, QC)],
                    start=True,
                    stop=True,
                )
                nc.tensor.matmul(
                    sB,
                    lhsT=kT_sb[p][DH:P, ts(kt, P)],
                    rhs=qT_sb[p][DH:P, ts(qc, QC)],
                    start=True,
                    stop=True,
                )
                eA = expp.tile([P, QC], BF, name=f"eA{p}_{qc}_{kt}", tag="exp")
                nc.scalar.activation(out=eA, in_=sA, func=EXP, scale=ACT_SCALE)
                eB = expp.tile([P, QC], BF, name=f"eB{p}_{qc}_{kt}", tag="exp")
                nc.vector._custom_dve(
                    EXP_OP,
                    out=eB.bitcast(U16),
                    in0=sB,
                    in1=kconst_sb,
                    s0=EXP_H,
                    s1=EXP_MAGIC,
                    imm2=EXP_C2,
                )
                e_tiles[kt] = (eA, eB)
                if kt >= AV_LAG:
                    emit_av(kt - AV_LAG)
            for kt in range(KT - AV_LAG, KT):
                emit_av(kt)
            if p == MT - 1 and qc == NQC - 1:
                # final q-chunk: latency-optimized path (this chain gates the
                # last out-proj tile, i.e. the kernel tail). Copies run on two
                # engines in parallel (bf16 out), the denominator partition
                # swap runs as a PE shift-matmul (no multi-us DGE latency),
                # and normalization runs on DVE (idle by then).
                stA = stagep.tile([P, QC], BF, name=f"stA{p}_{qc}", tag="st")
                nc.scalar.copy(out=stA, in_=psA)
                stB = stagep.tile([P, QC], BF, name=f"stB{p}_{qc}", tag="st")
                nc.vector.tensor_copy(out=stB, in_=psB)
                den_ps = ps_proj.tile([P, QC], F32, name=f"denps{p}", tag="ps_proj")
                nc.tensor.matmul(
                    den_ps[0:DH, :], lhsT=eyeshift_sb[:, 0:DH], rhs=stA,
                    start=True, stop=True,
                )
                nc.tensor.matmul(
                    den_ps[DH:P, :], lhsT=eyeshift_sb[:, DH:P], rhs=stB,
                    start=True, stop=True,
                )
                rec = recp.tile([P, QC], F32, name=f"rec{p}_{qc}", tag="rec")
                nc.vector.reciprocal_approx_fast(out=rec, in_=den_ps)
                nc.vector.tensor_mul(
                    out=ctx_sb[p][0:DH, ts(qc, QC)], in0=stA[0:DH], in1=rec[0:DH]
                )
                nc.vector.tensor_mul(
                    out=ctx_sb[p][DH:P, ts(qc, QC)], in0=stB[DH:P], in1=rec[DH:P]
                )
            else:
                # fast evacuation: free the PSUM pair with two plain copies so
                # the next q-chunk's AV matmuls never head-of-line-block PE.
                # ScalarE owns these (copy shares the exp activation table);
                # DVE's budget goes to its half of the exp tiles.
                stA = stagep.tile([P, QC], F32, name=f"stA{p}_{qc}", tag="st")
                nc.scalar.copy(out=stA, in_=psA)
                stB = stagep.tile([P, QC], F32, name=f"stB{p}_{qc}", tag="st")
                nc.scalar.copy(out=stB, in_=psB)
                # stA = [ctxA @0:64 | denA @64:128]; stB = [denB | ctxB]:
                # both heads' denominators gather into one reciprocal tile
                den = recp.tile([P, QC], F32, name=f"den{p}_{qc}", tag="den")
                nc.sync.dma_start(out=den[0:DH], in_=stA[DH:P])
                nc.sync.dma_start(out=den[DH:P], in_=stB[0:DH])
                rec = recp.tile([P, QC], F32, name=f"rec{p}_{qc}", tag="rec")
                nc.vector.reciprocal_approx_fast(out=rec, in_=den)
                nc.gpsimd.tensor_mul(
                    out=ctx_sb[p][0:DH, ts(qc, QC)], in0=stA[0:DH], in1=rec[0:DH]
                )
                nc.gpsimd.tensor_mul(
                    out=ctx_sb[p][DH:P, ts(qc, QC)], in0=stB[DH:P], in1=rec[DH:P]
                )
            if on_qc_done is not None:
                on_qc_done(qc)

    # ---- output projection -------------------------------------------
    part_sb = {}

    def emit_out_proj_partial(qc):
        # contract feature tiles 0..2 for this q-chunk early (their ctx
        # tiles finish before the last pair does); the tail then needs
        # only the k=3 matmul.
        for m in range(MT):
            ps = ps_proj.tile([P, QC], F32, name=f"pps{m}_{qc}", tag="ps_proj")
            for k in range(MT - 1):
                nc.tensor.matmul(
                    ps,
                    lhsT=w_sb["wo"][:, k, ts(m, P)],
                    rhs=ctx_sb[k][:, ts(qc, QC)],
                    start=(k == 0),
                    stop=(k == MT - 2),
                )
            pt = outp.tile([P, QC], F32, name=f"part{m}_{qc}", tag="part")
            nc.vector.tensor_copy(out=pt, in_=ps)
            part_sb[m] = pt

    def emit_out_proj_final(qc):
        for m in range(MT):
            ps = ps_proj.tile([P, QC], F32, name=f"ofin{m}_{qc}", tag="ps_proj")
            nc.tensor.matmul(
                ps,
                lhsT=w_sb["wo"][:, MT - 1, ts(m, P)],
                rhs=ctx_sb[MT - 1][:, ts(qc, QC)],
                start=True,
                stop=True,
            )
            o_sb = outp.tile([P, QC], F32, name=f"of{m}_{qc}", tag="out")
            nc.vector.scalar_tensor_tensor(
                out=o_sb,
                in0=ps,
                scalar=b_sb["bob"][:, m : m + 1],
                in1=part_sb[m],
                op0=ADD,
                op1=ADD,
            )
            nc.sync.dma_start(out=io["outT"][:, m, ts(qc, QC)], in_=o_sb)

    def emit_out_proj(qc):
        for m in range(MT):
            ps = ps_proj.tile([P, QC], F32, name=f"ops{m}_{qc}", tag="ps_proj")
            for k in range(MT):
                nc.tensor.matmul(
                    ps,
                    lhsT=w_sb["wo"][:, k, ts(m, P)],
                    rhs=ctx_sb[k][:, ts(qc, QC)],
                    start=(k == 0),
                    stop=(k == MT - 1),
                )
            o_sb = outp.tile([P, QC], F32, name=f"o{m}_{qc}", tag="out")
            nc.vector.tensor_scalar(
                out=o_sb,
                in0=ps,
                scalar1=b_sb["bob"][:, m : m + 1],
                scalar2=None,
                op0=ADD,
            )
            nc.sync.dma_start(out=io["outT"][:, m, ts(qc, QC)], in_=o_sb)

    # emission order: attention (exp-bound) starts as early as possible;
    # remaining projections backfill TensorE while exps stream.
    emit_qk_proj(0, qcs=[0])
    emit_v_proj(tts=range(0, 2))

    def qk_ahead(m_next, qc, kt):
        # spread the next pair's q/k' projection through this pair's last
        # q-chunk instead of a serial burst at the pair boundary
        if qc == NQC - 1 and kt in (2, 6, 10, 14):
            emit_qk_proj(m_next, qcs=[(kt - 2) // 4])

    def p0_hook(qc, kt):
        # pair-0 runs while inputs still stream in: emit the remaining
        # projections just-in-time so early scores/exps aren't scheduled
        # behind load-gated work.
        if qc == 0:
            tt = kt + 2
            if tt < KT:
                emit_v_proj(tts=[tt])
            if kt == 1:
                emit_qk_proj(0, qcs=[1])
            if kt == 5:
                emit_qk_proj(0, qcs=[2])
            if kt == 9:
                emit_qk_proj(0, qcs=[3])
        qk_ahead(1, qc, kt)

    def qk_ahead_spread(m_next, qc, kt):
        # one q-chunk of the next pair's projection per q-chunk of this
        # pair, so the DVE/ScalarE drain load spreads evenly
        if kt == 8:
            emit_qk_proj(m_next, qcs=[qc])

    emit_attention(0, pre_kt=p0_hook)
    emit_attention(1, pre_kt=lambda qc, kt: qk_ahead_spread(2, qc, kt))
    emit_attention(2, pre_kt=lambda qc, kt: qk_ahead_spread(3, qc, kt))

    def out_proj_hook(qc):
        if qc < NQC - 1:
            emit_out_proj(qc)
        if qc == NQC - 2:
            emit_out_proj_partial(NQC - 1)
        if qc == NQC - 1:
            emit_out_proj_final(NQC - 1)

    emit_attention(MT - 1, on_qc_done=out_proj_hook)


_CACHED_NC = None


def build_nc():
    global _CACHED_NC
    if _CACHED_NC is not None:
        return _CACHED_NC
    nc = bacc.Bacc("TRN2", target_bir_lowering=False, debug=False, num_devices=N_CORES)
    io = {}
    io["xT"] = nc.dram_tensor("xT", [P, MT, T], BF, kind="ExternalInput").ap()
    io["posT"] = nc.dram_tensor("posT", [P, MT, T], BF, kind="ExternalInput").ap()
    for wname in ("wq", "wk", "wp", "wv", "wo"):
        io[wname + "T"] = nc.dram_tensor(
            wname + "T", [P, MT, D], BF, kind="ExternalInput"
        ).ap()
    for bname in ("bq", "bkp", "bob"):
        io[bname] = nc.dram_tensor(bname, [P, MT], F32, kind="ExternalInput").ap()
    io["eyeshift"] = nc.dram_tensor("eyeshift", [P, P], BF, kind="ExternalInput").ap()
    io["outT"] = nc.dram_tensor("outT", [P, MT, T], F32, kind="ExternalOutput").ap()

    from contextlib import ExitStack

    with tile.TileContext(nc) as tc, ExitStack() as ctx:
        _emit(ctx, tc, io)
    nc.compile()
    _CACHED_NC = nc
    return nc


def _to_bf16(a):
    return np.asarray(a, dtype=np.float32).astype(ml_dtypes.bfloat16)


def _retile(mat):
    # [R, C] with R = MT*P  ->  [P, MT, C]
    r, c = mat.shape
    return np.ascontiguousarray(mat.reshape(MT, P, c).transpose(1, 0, 2))


def make_in_maps(x, pos_embeddings, Wq, bq, Wk, bk, Wv, bv, Wp, bp, Wo, bo):
    """Host-side prep: transpose / retile / fold biases / cast to bf16."""
    x = np.asarray(x, np.float32)
    pos = np.asarray(pos_embeddings, np.float32)
    wqT = _to_bf16(_retile(np.asarray(Wq, np.float32).T))
    wkT = _to_bf16(_retile(np.asarray(Wk, np.float32).T))
    wpT = _to_bf16(_retile(np.asarray(Wp, np.float32).T))
    wvT = _to_bf16(_retile(np.asarray(Wv, np.float32).T))
    woT = _to_bf16(_retile(np.asarray(Wo, np.float32).T))
    bq_t = np.ascontiguousarray(np.asarray(bq, np.float32).reshape(MT, P).T)
    bkp = np.ascontiguousarray(
        (np.asarray(bk, np.float32) + np.asarray(bp, np.float32)).reshape(MT, P).T
    )
    bob = np.ascontiguousarray(
        (
            np.asarray(bo, np.float32)
            + np.asarray(Wo, np.float32) @ np.asarray(bv, np.float32)
        ).reshape(MT, P).T
    )
    eye = np.zeros((P, P), np.float32)
    idx = np.arange(DH)
    eye[idx + DH, idx] = 1.0  # den_even[j] <- stA[j+64]
    eye[idx, idx + DH] = 1.0  # den_odd[64+j] <- stB[j]
    eyeshift = eye.astype(ml_dtypes.bfloat16)

    in_maps = []
    for b in range(B):
        xT = _to_bf16(_retile(np.ascontiguousarray(x[b].T)))
        posT = _to_bf16(_retile(np.ascontiguousarray(pos[b].T)))
        in_maps.append(
            dict(
                xT=xT,
                posT=posT,
                wqT=wqT,
                wkT=wkT,
                wpT=wpT,
                wvT=wvT,
                woT=woT,
                bq=bq_t,
                bkp=bkp,
                bob=bob,
                eyeshift=eyeshift,
            )
        )
    return in_maps


def assemble_output(results):
    out = np.empty((B, T, D), np.float32)
    for b in range(B):
        # outT [P, MT, T] -> [D, T] -> [T, D]
        out[b] = results[b]["outT"].transpose(1, 0, 2).reshape(D, T).T
    return out


def kernel(**inputs) -> np.ndarray:
    nc = build_nc()
    in_maps = make_in_maps(**inputs)
    res = run_bass_kernel_spmd(nc, in_maps, core_ids=list(range(N_CORES)))
    return assemble_output(res.results)


if __name__ == "__main__":
    import reference

    inputs = {k: np.asarray(v) for k, v in reference.setup_inputs().items()}
    got = kernel(**inputs)
    exp = np.asarray(reference.reference(**inputs))
    err = np.abs(got - exp)
    rel = np.linalg.norm(got - exp) / np.linalg.norm(exp)
    print("max abs err:", err.max(), "rel:", rel)


# revision 27
# speedup vs baseline: 1.0170x; 1.0170x over previous
"""Trainium2 Bass kernel for dual-score multi-head self-attention.

Reference computation (per batch b):
    q   = x @ Wq.T + bq          k = x @ Wk.T + bk
    v   = x @ Wv.T + bv          pos = pos_emb @ Wp.T + bp
    scores = (q k^T + q pos^T) / sqrt(dh)   (per head)
    out = softmax(scores) @ v, merged heads, @ Wo.T + bo

Algebraic folds:
  * q k^T + q pos^T == q (k+pos)^T; k+pos accumulates in one PSUM tile.
  * v's bias commutes through softmax (rows sum to 1): bo' = bo + Wo @ bv.
  * softmax denominators ride along in the attention*V matmul via ones
    columns appended to V (even heads [v|ones], odd heads [ones|v] so
    every DVE op in the normalization runs at a legal base partition).
  * q is pre-scaled by ALPHA = 128*log2(e)/8 so scores arrive in
    "128-scaled log2" units: the DVE exp tiles consume them directly via
    a fused custom op (corrected Schraudolph), the ScalarE exp tiles undo
    the scale inside the activation's scale factor.

Exp is split across ScalarE (activation table) and DVE (custom fused op:
bits = c2*(|frac_128(s)| - h)^2 + s + K, rounded into bf16's u16 bit
pattern) to break the ScalarE bottleneck; softmax normalization muls run
on GPSIMD.

Sharding: batch parallel, one batch per core, weights replicated, no
collectives. See bottom for host-side prep.
"""

import numpy as np

try:
    import concourse.bass as bass  # noqa: F401
except ImportError:  # pragma: no cover - container default path
    import sys

    for p in ("/opt/trn_rl_repo", "/root/.axon_site/_ro/trn_rl_repo"):
        if p not in sys.path:
            sys.path.insert(0, p)

import ml_dtypes

import concourse.bass as bass
import concourse.mybir as mybir
import concourse.tile as tile
import concourse.dve_ops as dve_ops
from concourse import bacc
from concourse.bass import ts
from concourse.bass_utils import run_bass_kernel_spmd
from concourse.dve_spec import Spec, Src0, Src1, C0, C1, C2, lower, Bin, AluOp, sq
from concourse.dve_ops import DveOp, OPS, DveOpSpec, has_src1

P = 128
T = 2048
D = 512
H = 8
DH = 64
B = 8
MT = D // P  # 4 feature tiles
KT = T // P  # 16 kpos tiles
QC = 512  # q chunk (one PSUM bank of f32)
NQC = T // QC

BF = mybir.dt.bfloat16
F32 = mybir.dt.float32
U16 = mybir.dt.uint16
ADD = mybir.AluOpType.add
MULT = mybir.AluOpType.mult
EXP = mybir.ActivationFunctionType.Exp

N_CORES = 8

# ---------------------------------------------------------------------------
# Custom DVE exp op: corrected Schraudolph.
#   in0 = s' (scores pre-scaled by ALPHA), out bits (u16, reinterpreted bf16)
#   bits = c2*(|s' - RN_128(s')| - h)^2 + (s' + K)
# Calibrated: rms rel err 0.25%, max 0.55% vs exp (incl. bf16 rounding).
EXP_H = 66.124491335875774
EXP_C2 = 0.0025442885269545651
EXP_K = 16244.943801046942
EXP_MAGIC = float(1.5 * 2.0**30)
ALPHA = float(128.0 * np.log2(np.e) / 8.0)  # q pre-scale per raw score unit
ACT_SCALE = float(1.0 / (8.0 * ALPHA))  # ScalarE: exp(s' * ACT_SCALE)

# Exp tiles are split per head: head-even half on ScalarE, head-odd half
# on the DVE custom op, concurrently. Halving the exp instruction length
# halves the score->AV latency, which is what lets the lag-2 software
# pipeline below run gapless on the PE.


def _exp_ref(in0, in1, s0, s1, imm2):
    in0 = in0.astype(np.float32)
    w = (in0 + np.float32(s1)).astype(np.float32)
    r = (w - np.float32(s1)).astype(np.float32)
    af = np.abs(in0 - r).astype(np.float32)
    d = (af - np.float32(s0)).astype(np.float32)
    return ((d * d * np.float32(imm2)) + (in0 + in1)).astype(np.float32)


def _register_exp_op():
    for op in OPS:
        if op.name == "EXP_SCH_ANT":
            return op
    w = Src0 + C1
    r = w - C1
    af = Bin(AluOp.ABSOLUTE_DIFF, Src0, r)
    d = af - C0
    body = sq(d) * C2 + (Src0 + Src1)
    spec = Spec(body=body, reference=_exp_ref)
    op = DveOp("EXP_SCH_ANT", spec, subdim=False, uops_sha={})
    OPS.append(op)
    dve_ops._SUB_OPCODE_FOR_NAME[op.name] = dve_ops._CUSTOM_DVE_ROW_BASE + len(OPS) - 1
    dve_ops.CUSTOM_DVE_SPECS[op.name] = spec
    opc = dve_ops.get_dve_sub_opcode(op.name)
    for ver in ("v3", "v4"):
        uops = lower(spec, ver=ver)
        comp = DveOpSpec(name=op.name, opcode=opc, uops=uops, rd1_en=has_src1(spec))
        op.uops_sha[ver] = comp.sha(ver)
    return op


EXP_OP = _register_exp_op()


def _emit(ctx, tc, io):
    nc = tc.nc

    # ---- persistent SBUF tensors -------------------------------------
    const_pool = ctx.enter_context(tc.tile_pool(name="const", bufs=1))

    def single(name, shape, dtype):
        return const_pool.tile(shape, dtype, name=name, tag=name)

    w_sb = {w: single(f"w_{w}", [P, MT, D], BF) for w in
            ("wq", "wk", "wp", "wv", "wo")}
    b_sb = {b: single(f"b_{b}", [P, MT], F32) for b in ("bq", "bkp", "bob")}
    xT_sb = single("xT_sb", [P, MT, T], BF)
    posT_sb = single("posT_sb", [P, MT, T], BF)
    qT_sb = [single(f"qT{m}", [P, T], BF) for m in range(MT)]
    kT_sb = [single(f"kT{m}", [P, T], BF) for m in range(MT)]
    ctx_sb = [single(f"ctxT{m}", [P, T], BF) for m in range(MT)]
    v_sb = [single(f"v{kt}", [P, H, P], BF) for kt in range(KT)]
    # elementwise K operand for the custom DVE exp ([P,1] broadcast Src1
    # wedges the DVE on this stack, so spend the SBUF on a full tile)
    kconst_sb = single("kconst", [P, QC], F32)
    # partition-swap matrix (i <-> i+64) for the tail's PE-based
    # denominator extraction
    eyeshift_sb = single("eyeshift", [P, P], BF)

    # ---- input DMAs: one transfer per tensor (chunked along T for x/pos
    # so compute starts early); four DGE queues carry ~equal bytes in
    # first-need order (q-proj -> k'-proj -> v-proj -> rest).
    nc.sync.dma_start(out=xT_sb[:, :, 0:QC], in_=io["xT"][:, :, 0:QC])
    nc.scalar.dma_start(out=w_sb["wq"], in_=io["wqT"])
    nc.gpsimd.dma_start(out=posT_sb[:, :, 0:QC], in_=io["posT"][:, :, 0:QC])
    nc.scalar.dma_start(out=w_sb["wk"], in_=io["wkT"])
    nc.sync.dma_start(out=w_sb["wp"], in_=io["wpT"])
    nc.gpsimd.dma_start(out=w_sb["wv"], in_=io["wvT"])
    nc.scalar.dma_start(out=b_sb["bq"], in_=io["bq"])
    nc.scalar.dma_start(out=b_sb["bkp"], in_=io["bkp"])
    nc.sync.dma_start(out=xT_sb[:, :, QC:T], in_=io["xT"][:, :, QC:T])
    nc.scalar.dma_start(out=posT_sb[:, :, QC:T], in_=io["posT"][:, :, QC:T])
    nc.sync.dma_start(out=w_sb["wo"], in_=io["woT"])
    nc.scalar.dma_start(out=b_sb["bob"], in_=io["bob"])
    nc.scalar.dma_start(out=eyeshift_sb, in_=io["eyeshift"])

    nc.gpsimd.memset(kconst_sb, EXP_K)
    for kt in range(KT):
        # only the ones-regions: keeps the v drains (disjoint slices)
        # independent of the memsets under subtile dep tracking
        nc.gpsimd.memset(v_sb[kt][:, 0 : H : 2, DH:P], 1.0)
        nc.gpsimd.memset(v_sb[kt][:, 1 : H : 2, 0:DH], 1.0)

    # ---- pools --------------------------------------------------------
    ps_proj = ctx.enter_context(tc.tile_pool(name="ps_proj", bufs=2, space="PSUM"))
    ps_sc = ctx.enter_context(tc.tile_pool(name="ps_sc", bufs=4, space="PSUM"))
    ps_av = ctx.enter_context(tc.tile_pool(name="ps_av", bufs=2, space="PSUM"))
    expp = ctx.enter_context(tc.tile_pool(name="expp", bufs=8))
    recp = ctx.enter_context(tc.tile_pool(name="recp", bufs=4))
    stagep = ctx.enter_context(tc.tile_pool(name="stagep", bufs=4))
    outp = ctx.enter_context(tc.tile_pool(name="outp", bufs=6))

    # prime the ScalarE exp table set (~2.7us one-time load) during the
    # DMA phase instead of on the first real score tile
    warm = recp.tile([P, 1], F32, name="warm", tag="warm")
    nc.vector.memset(warm, 0.0)
    nc.scalar.activation(out=warm, in_=warm, func=EXP, scale=1.0)

    # ---- v projection (natural layout, no bias) ----------------------
    def emit_v_proj(tts=None):
        for tt in tts if tts is not None else range(KT):
            ps = ps_proj.tile([P, D], F32, name=f"vps{tt}", tag="ps_proj")
            for k in range(MT):
                nc.tensor.matmul(
                    ps,
                    lhsT=xT_sb[:, k, ts(tt, P)],
                    rhs=w_sb["wv"][:, k, :],
                    start=(k == 0),
                    stop=(k == MT - 1),
                )
            ps_h = ps.rearrange("p (h d) -> p h d", h=H)
            nc.vector.tensor_copy(out=v_sb[tt][:, 0:H:2, 0:DH], in_=ps_h[:, 0:H:2, :])
            nc.vector.tensor_copy(out=v_sb[tt][:, 1:H:2, DH:P], in_=ps_h[:, 1:H:2, :])

    # ---- q / k' projections for one feature tile m -------------------
    # q is scaled by ALPHA so the score matmul emits pre-scaled scores.
    def emit_qk_proj(m, qcs=None):
        for qc in qcs if qcs is not None else range(NQC):
            ps = ps_proj.tile([P, QC], F32, name=f"qps{m}_{qc}", tag="ps_proj")
            for k in range(MT):
                nc.tensor.matmul(
                    ps,
                    lhsT=w_sb["wq"][:, k, ts(m, P)],
                    rhs=xT_sb[:, k, ts(qc, QC)],
                    start=(k == 0),
                    stop=(k == MT - 1),
                )
            nc.vector.tensor_scalar(
                out=qT_sb[m][:, ts(qc, QC)],
                in0=ps,
                scalar1=b_sb["bq"][:, m : m + 1],
                scalar2=ALPHA,
                op0=ADD,
                op1=MULT,
            )
            ps2 = ps_proj.tile([P, QC], F32, name=f"kps{m}_{qc}", tag="ps_proj")
            for k in range(MT):
                nc.tensor.matmul(
                    ps2,
                    lhsT=w_sb["wk"][:, k, ts(m, P)],
                    rhs=xT_sb[:, k, ts(qc, QC)],
                    start=(k == 0),
                    stop=False,
                )
            for k in range(MT):
                nc.tensor.matmul(
                    ps2,
                    lhsT=w_sb["wp"][:, k, ts(m, P)],
                    rhs=posT_sb[:, k, ts(qc, QC)],
                    start=False,
                    stop=(k == MT - 1),
                )
            nc.vector.tensor_scalar(
                out=kT_sb[m][:, ts(qc, QC)],
                in0=ps2,
                scalar1=b_sb["bkp"][:, m : m + 1],
                scalar2=None,
                op0=ADD,
            )

    # ---- attention for head pair p (heads 2p, 2p+1) ------------------
    # Lag-2 software pipeline: the AV pair for kt is emitted two score
    # iterations later, giving the exps (ScalarE half + DVE half, run
    # concurrently) two full PE slots of latency headroom.
    AV_LAG = 2

    def emit_attention(p, on_qc_done=None, pre_kt=None):
        for qc in range(NQC):
            psA = ps_av.tile([P, QC], F32, name=f"avA{p}_{qc}", tag="av")
            psB = ps_av.tile([P, QC], F32, name=f"avB{p}_{qc}", tag="av")
            e_tiles = {}

            def emit_av(kt):
                eA, eB = e_tiles.pop(kt)
                nc.tensor.matmul(
                    psA,
                    lhsT=v_sb[kt][:, 2 * p, :],
                    rhs=eA,
                    start=(kt == 0),
                    stop=(kt == KT - 1),
                    skip_group_check=True,
                )
                nc.tensor.matmul(
                    psB,
                    lhsT=v_sb[kt][:, 2 * p + 1, :],
                    rhs=eB,
                    start=(kt == 0),
                    stop=(kt == KT - 1),
                    skip_group_check=True,
                )

            # Groups of two kt per round: the consecutive AV streams let the
            # second AV pair's (full-row) weight loads hide under in-flight
            # full-row streams; only the group-leading loads stay exposed.
            def emit_sc_exp(kt):
                if pre_kt is not None:
                    pre_kt(qc, kt)
                sA = ps_sc.tile([P, QC], F32, name=f"scA{p}_{qc}_{kt}", tag="sc")
                sB = ps_sc.tile([P, QC], F32, name=f"scB{p}_{qc}_{kt}", tag="sc")
                # scores^T = k'^T.T @ q^T, two heads packed via row tiling
                nc.tensor.matmul(
                    sA,
                    lhsT=kT_sb[p][0:DH, ts(kt, P)],
                    rhs=qT_sb[p][0:DH, ts(qc, QC)],
                    start=True,
                    stop=True,
                )
                nc.tensor.matmul(
                    sB,
                    lhsT=kT_sb[p][DH:P, ts(kt, P)],
                    rhs=qT_sb[p][DH:P, ts(qc, QC)],
                    start=True,
                    stop=True,
                )
                eA = expp.tile([P, QC], BF, name=f"eA{p}_{qc}_{kt}", tag="exp")
                nc.scalar.activation(out=eA, in_=sA, func=EXP, scale=ACT_SCALE)
                eB = expp.tile([P, QC], BF, name=f"eB{p}_{qc}_{kt}", tag="exp")
                nc.vector._custom_dve(
                    EXP_OP,
                    out=eB.bitcast(U16),
                    in0=sB,
                    in1=kconst_sb,
                    s0=EXP_H,
                    s1=EXP_MAGIC,
                    imm2=EXP_C2,
                )
                e_tiles[kt] = (eA, eB)

            for g in range(0, KT, 2):
                emit_sc_exp(g)
                emit_sc_exp(g + 1)
                if g >= AV_LAG:
                    emit_av(g - AV_LAG)
                    emit_av(g - AV_LAG + 1)
            for kt in range(KT - AV_LAG, KT):
                emit_av(kt)
            if p == MT - 1 and qc == NQC - 1:
                # final q-chunk: latency-optimized path (this chain gates the
                # last out-proj tile, i.e. the kernel tail). Copies run on two
                # engines in parallel (bf16 out), the denominator partition
                # swap runs as a PE shift-matmul (no multi-us DGE latency),
                # and normalization runs on DVE (idle by then).
                stA = stagep.tile([P, QC], BF, name=f"stA{p}_{qc}", tag="st")
                nc.scalar.copy(out=stA, in_=psA)
                stB = stagep.tile([P, QC], BF, name=f"stB{p}_{qc}", tag="st")
                nc.vector.tensor_copy(out=stB, in_=psB)
                den_ps = ps_proj.tile([P, QC], F32, name=f"denps{p}", tag="ps_proj")
                nc.tensor.matmul(
                    den_ps[0:DH, :], lhsT=eyeshift_sb[:, 0:DH], rhs=stA,
                    start=True, stop=True,
                )
                nc.tensor.matmul(
                    den_ps[DH:P, :], lhsT=eyeshift_sb[:, DH:P], rhs=stB,
                    start=True, stop=True,
                )
                rec = recp.tile([P, QC], F32, name=f"rec{p}_{qc}", tag="rec")
                nc.vector.reciprocal_approx_fast(out=rec, in_=den_ps)
                nc.vector.tensor_mul(
                    out=ctx_sb[p][0:DH, ts(qc, QC)], in0=stA[0:DH], in1=rec[0:DH]
                )
                nc.vector.tensor_mul(
                    out=ctx_sb[p][DH:P, ts(qc, QC)], in0=stB[DH:P], in1=rec[DH:P]
                )
            else:
                # fast evacuation: free the PSUM pair with two plain copies so
                # the next q-chunk's AV matmuls never head-of-line-block PE.
                # ScalarE owns these (copy shares the exp activation table);
                # DVE's budget goes to its half of the exp tiles.
                stA = stagep.tile([P, QC], F32, name=f"stA{p}_{qc}", tag="st")
                nc.scalar.copy(out=stA, in_=psA)
                stB = stagep.tile([P, QC], F32, name=f"stB{p}_{qc}", tag="st")
                nc.scalar.copy(out=stB, in_=psB)
                # stA = [ctxA @0:64 | denA @64:128]; stB = [denB | ctxB]:
                # both heads' denominators gather into one reciprocal tile
                den = recp.tile([P, QC], F32, name=f"den{p}_{qc}", tag="den")
                nc.sync.dma_start(out=den[0:DH], in_=stA[DH:P])
                nc.sync.dma_start(out=den[DH:P], in_=stB[0:DH])
                rec = recp.tile([P, QC], F32, name=f"rec{p}_{qc}", tag="rec")
                nc.vector.reciprocal_approx_fast(out=rec, in_=den)
                nc.gpsimd.tensor_mul(
                    out=ctx_sb[p][0:DH, ts(qc, QC)], in0=stA[0:DH], in1=rec[0:DH]
                )
                nc.gpsimd.tensor_mul(
                    out=ctx_sb[p][DH:P, ts(qc, QC)], in0=stB[DH:P], in1=rec[DH:P]
                )
            if on_qc_done is not None:
                on_qc_done(qc)

    # ---- output projection -------------------------------------------
    part_sb = {}

    def emit_out_proj_partial(qc):
        # contract feature tiles 0..2 for this q-chunk early (their ctx
        # tiles finish before the last pair does); the tail then needs
        # only the k=3 matmul.
        for m in range(MT):
            ps = ps_proj.tile([P, QC], F32, name=f"pps{m}_{qc}", tag="ps_proj")
            for k in range(MT - 1):
                nc.tensor.matmul(
                    ps,
                    lhsT=w_sb["wo"][:, k, ts(m, P)],
                    rhs=ctx_sb[k][:, ts(qc, QC)],
                    start=(k == 0),
                    stop=(k == MT - 2),
                )
            pt = outp.tile([P, QC], F32, name=f"part{m}_{qc}", tag="part")
            nc.vector.tensor_copy(out=pt, in_=ps)
            part_sb[m] = pt

    def emit_out_proj_final(qc):
        for m in range(MT):
            ps = ps_proj.tile([P, QC], F32, name=f"ofin{m}_{qc}", tag="ps_proj")
            nc.tensor.matmul(
                ps,
                lhsT=w_sb["wo"][:, MT - 1, ts(m, P)],
                rhs=ctx_sb[MT - 1][:, ts(qc, QC)],
                start=True,
                stop=True,
            )
            o_sb = outp.tile([P, QC], F32, name=f"of{m}_{qc}", tag="out")
            nc.vector.scalar_tensor_tensor(
                out=o_sb,
                in0=ps,
                scalar=b_sb["bob"][:, m : m + 1],
                in1=part_sb[m],
                op0=ADD,
                op1=ADD,
            )
            nc.sync.dma_start(out=io["outT"][:, m, ts(qc, QC)], in_=o_sb)

    def emit_out_proj(qc):
        for m in range(MT):
            ps = ps_proj.tile([P, QC], F32, name=f"ops{m}_{qc}", tag="ps_proj")
            for k in range(MT):
                nc.tensor.matmul(
                    ps,
                    lhsT=w_sb["wo"][:, k, ts(m, P)],
                    rhs=ctx_sb[k][:, ts(qc, QC)],
                    start=(k == 0),
                    stop=(k == MT - 1),
                )
            o_sb = outp.tile([P, QC], F32, name=f"o{m}_{qc}", tag="out")
            nc.vector.tensor_scalar(
                out=o_sb,
                in0=ps,
                scalar1=b_sb["bob"][:, m : m + 1],
                scalar2=None,
                op0=ADD,
            )
            nc.sync.dma_start(out=io["outT"][:, m, ts(qc, QC)], in_=o_sb)

    # emission order: attention (exp-bound) starts as early as possible;
    # remaining projections backfill TensorE while exps stream.
    emit_qk_proj(0, qcs=[0])
    emit_v_proj(tts=range(0, 2))

    def qk_ahead(m_next, qc, kt):
        # spread the next pair's q/k' projection through this pair's last
        # q-chunk instead of a serial burst at the pair boundary
        if qc == NQC - 1 and kt in (2, 6, 10, 14):
            emit_qk_proj(m_next, qcs=[(kt - 2) // 4])

    def p0_hook(qc, kt):
        # pair-0 runs while inputs still stream in: emit the remaining
        # projections just-in-time so early scores/exps aren't scheduled
        # behind load-gated work.
        if qc == 0:
            tt = kt + 2
            if tt < KT:
                emit_v_proj(tts=[tt])
            if kt == 1:
                emit_qk_proj(0, qcs=[1])
            if kt == 5:
                emit_qk_proj(0, qcs=[2])
            if kt == 9:
                emit_qk_proj(0, qcs=[3])
        qk_ahead(1, qc, kt)

    def qk_ahead_spread(m_next, qc, kt):
        # one q-chunk of the next pair's projection per q-chunk of this
        # pair, so the DVE/ScalarE drain load spreads evenly
        if kt == 8:
            emit_qk_proj(m_next, qcs=[qc])

    emit_attention(0, pre_kt=p0_hook)
    emit_attention(1, pre_kt=lambda qc, kt: qk_ahead_spread(2, qc, kt))
    emit_attention(2, pre_kt=lambda qc, kt: qk_ahead_spread(3, qc, kt))

    def out_proj_hook(qc):
        if qc < NQC - 1:
            emit_out_proj(qc)
        if qc == NQC - 2:
            emit_out_proj_partial(NQC - 1)
        if qc == NQC - 1:
            emit_out_proj_final(NQC - 1)

    emit_attention(MT - 1, on_qc_done=out_proj_hook)


_CACHED_NC = None


def build_nc():
    global _CACHED_NC
    if _CACHED_NC is not None:
        return _CACHED_NC
    nc = bacc.Bacc("TRN2", target_bir_lowering=False, debug=False, num_devices=N_CORES)
    io = {}
    io["xT"] = nc.dram_tensor("xT", [P, MT, T], BF, kind="ExternalInput").ap()
    io["posT"] = nc.dram_tensor("posT", [P, MT, T], BF, kind="ExternalInput").ap()
    for wname in ("wq", "wk", "wp", "wv", "wo"):
        io[wname + "T"] = nc.dram_tensor(
            wname + "T", [P, MT, D], BF, kind="ExternalInput"
        ).ap()
    for bname in ("bq", "bkp", "bob"):
        io[bname] = nc.dram_tensor(bname, [P, MT], F32, kind="ExternalInput").ap()
    io["eyeshift"] = nc.dram_tensor("eyeshift", [P, P], BF, kind="ExternalInput").ap()
    io["outT"] = nc.dram_tensor("outT", [P, MT, T], F32, kind="ExternalOutput").ap()

    from contextlib import ExitStack

    with tile.TileContext(nc) as tc, ExitStack() as ctx:
        _emit(ctx, tc, io)
    nc.compile()
    _CACHED_NC = nc
    return nc


def _to_bf16(a):
    return np.asarray(a, dtype=np.float32).astype(ml_dtypes.bfloat16)


def _retile(mat):
    # [R, C] with R = MT*P  ->  [P, MT, C]
    r, c = mat.shape
    return np.ascontiguousarray(mat.reshape(MT, P, c).transpose(1, 0, 2))


def make_in_maps(x, pos_embeddings, Wq, bq, Wk, bk, Wv, bv, Wp, bp, Wo, bo):
    """Host-side prep: transpose / retile / fold biases / cast to bf16."""
    x = np.asarray(x, np.float32)
    pos = np.asarray(pos_embeddings, np.float32)
    wqT = _to_bf16(_retile(np.asarray(Wq, np.float32).T))
    wkT = _to_bf16(_retile(np.asarray(Wk, np.float32).T))
    wpT = _to_bf16(_retile(np.asarray(Wp, np.float32).T))
    wvT = _to_bf16(_retile(np.asarray(Wv, np.float32).T))
    woT = _to_bf16(_retile(np.asarray(Wo, np.float32).T))
    bq_t = np.ascontiguousarray(np.asarray(bq, np.float32).reshape(MT, P).T)
    bkp = np.ascontiguousarray(
        (np.asarray(bk, np.float32) + np.asarray(bp, np.float32)).reshape(MT, P).T
    )
    bob = np.ascontiguousarray(
        (
            np.asarray(bo, np.float32)
            + np.asarray(Wo, np.float32) @ np.asarray(bv, np.float32)
        ).reshape(MT, P).T
    )
    eye = np.zeros((P, P), np.float32)
    idx = np.arange(DH)
    eye[idx + DH, idx] = 1.0  # den_even[j] <- stA[j+64]
    eye[idx, idx + DH] = 1.0  # den_odd[64+j] <- stB[j]
    eyeshift = eye.astype(ml_dtypes.bfloat16)

    in_maps = []
    for b in range(B):
        xT = _to_bf16(_retile(np.ascontiguousarray(x[b].T)))
        posT = _to_bf16(_retile(np.ascontiguousarray(pos[b].T)))
        in_maps.append(
            dict(
                xT=xT,
                posT=posT,
                wqT=wqT,
                wkT=wkT,
                wpT=wpT,
                wvT=wvT,
                woT=woT,
                bq=bq_t,
                bkp=bkp,
                bob=bob,
                eyeshift=eyeshift,
            )
        )
    return in_maps


def assemble_output(results):
    out = np.empty((B, T, D), np.float32)
    for b in range(B):
        # outT [P, MT, T] -> [D, T] -> [T, D]
        out[b] = results[b]["outT"].transpose(1, 0, 2).reshape(D, T).T
    return out


def kernel(**inputs) -> np.ndarray:
    nc = build_nc()
    in_maps = make_in_maps(**inputs)
    res = run_bass_kernel_spmd(nc, in_maps, core_ids=list(range(N_CORES)))
    return assemble_output(res.results)


if __name__ == "__main__":
    import reference

    inputs = {k: np.asarray(v) for k, v in reference.setup_inputs().items()}
    got = kernel(**inputs)
    exp = np.asarray(reference.reference(**inputs))
    err = np.abs(got - exp)
    rel = np.linalg.norm(got - exp) / np.linalg.norm(exp)
    print("max abs err:", err.max(), "rel:", rel)
